# revision 52
# baseline (speedup 1.0000x reference)
# LongNetViT forward on 8 Trainium2 NeuronCores (Bass/Tile SPMD).
#
# Sharding: tokens are split 1024/core for embedding, layernorms, projections
# and FFN; the dilated attention of layer 1 is head-sharded (core c owns head c
# for every (segment, dilation) block).  q/k/v are projected LOCALLY (each core
# projects its own 1024 tokens for all 8 heads), then an AllToAll distributes
# per-head 32-feature slices (rotated per destination so every dilation class
# starts at column 0 with stride r on every core).  The score matmuls have
# contraction dim 32 (one head), so four 128-key chunks are packed into the
# 128x128 PE array concurrently via tile_position row-tiling, using 4x
# replicated q/k.  Per-head (num, den) softmax accumulators return via a
# second AllToAll.  Layer 2 only needs the cls row; each core computes
# flash-style partial softmax sums over its local keys and a tiny AllGather
# finishes the job.  Softmax is computed without max-subtraction (scores are
# O(1) here) so per-branch results fuse by plain summation of exp-sums.
import numpy as np
import ml_dtypes

BF = ml_dtypes.bfloat16
NCORES = 8
D_IN, D, H, HD = 1536, 256, 8, 32
FFN = 1024
B, L = 1, 8191
S = 8192
TPC = 1024          # tokens per core
NGRIDS, TILE_SZ = 256, 256
SEGMENTS = [1024, 2048, 4096, 8192, 16384]
RATIOS = [1, 2, 4, 8, 16]
SCALE = float(HD) ** -0.5

_CACHE = {}


# ----------------------------------------------------------------------------
# program builder
# ----------------------------------------------------------------------------
def build_program(debug=False):
    import concourse.bass as bass
    import concourse.mybir as mybir
    from concourse import bacc
    import concourse.tile as tile

    F32 = mybir.dt.float32
    BF16 = mybir.dt.bfloat16
    I32 = mybir.dt.int32
    AF = mybir.ActivationFunctionType
    ALU = mybir.AluOpType

    nc = bacc.Bacc("TRN2", target_bir_lowering=False, debug=False,
                   num_devices=NCORES)

    def din(name, shape, dtype=F32):
        return nc.dram_tensor(name, list(shape), dtype, kind="ExternalInput")

    # inputs (already laid out host-side exactly as SBUF wants them)
    xt_in = din("xt", [128, 12, TPC], BF16)       # feature-major x (host-transposed)
    crd_in = din("crd", [2, TPC], I32)
    tab_in = din("tab", [128, 2, 128], BF16)
    iota_in = din("iota2", [128, 2])
    id_in = din("id128", [128, 128])
    sel_in = din("selm", [8, 2, 128], BF16)
    hsel_in = din("hsel", [128, 2, 8])
    sel64_in = din("sel64", [64, 8])
    clsv_in = din("clsv", [1, 256], BF16)
    brow_in = din("brow", [1, TPC], BF16)
    pw_in = din("pw", [128, 12, 256], BF16)
    pb_in = din("pb", [1, 256], BF16)
    wq_in = din("wqf", [128, 2, 256], BF16)
    bq_in = din("bqc", [128, 2])
    wk_in = din("wkf", [128, 2, 256], BF16)
    bk_in = din("bkc", [128, 2])
    wv_in = din("wvf", [128, 2, 256], BF16)
    bv_in = din("bvc", [128, 2])
    wo_in = din("wo", [128, 2, 256], BF16)
    bo_in = din("boc", [128, 2])
    w1_in = din("w1", [128, 2, FFN], BF16)
    b1_in = din("b1c", [128, 8])
    w2_in = din("w2", [128, 8, 256], BF16)
    b2_in = din("b2c", [128, 2])
    wk2_in = din("wk2", [128, 2, 256], BF16)
    bk2_in = din("bk2c", [128, 2])
    wv2_in = din("wv2", [128, 2, 256], BF16)
    bv2_in = din("bv2c", [128, 2])
    wq2_in = din("wq2", [128, 2, 256])
    bq2_in = din("bq2c", [128, 2])
    wo2_in = din("wo2", [128, 2, 256])
    bo2_in = din("bo2c", [128, 2])
    w12_in = din("w12", [128, 2, FFN])
    b12_in = din("b12c", [128, 8])
    w22_in = din("w22", [128, 8, 256])
    b22_in = din("b22c", [128, 2])
    m2_in = din("m2r", [8, TPC], BF16)
    encg_in = din("encg", [128, 2])
    encb_in = din("encb", [128, 2])
    nrmg_in = din("nrmg", [128, 2])
    nrmb_in = din("nrmb", [128, 2])

    out_d = nc.dram_tensor("out", [2, 128], F32, kind="ExternalOutput")
    dbg = {}
    if debug:
        for nm, shp, dt_ in [
                ("dbg_h0", [128, 2048], F32), ("dbg_xh", [128, 2048], F32),
                ("dbg_q", [128, 8192], BF16), ("dbg_k", [128, 8192], BF16),
                ("dbg_v", [32, 8192], BF16), ("dbg_acc", [128, 4096], F32),
                ("dbg_att", [128, 2048], F32), ("dbg_h1", [128, 2048], F32),
                ("dbg_h2", [128, 2048], F32)]:
            dbg[nm] = nc.dram_tensor(nm, shp, dt_, kind="ExternalOutput")

    RG = [[i for i in range(NCORES)]]

    with tile.TileContext(nc) as tc:
        with tc.tile_pool(name="wpool", bufs=1) as wp, \
             tc.tile_pool(name="mainp", bufs=1) as mp, \
             tc.tile_pool(name="dramp", bufs=1, space="DRAM") as dp:

            # ---- persistent weights/consts -------------------------------
            _rr = [0]

            def wtile(src, shape, dt_=F32):
                t = wp.tile(shape, dt_, name=src.name + "_sb")
                eng = nc.sync if _rr[0] % 2 == 0 else nc.scalar
                _rr[0] += 1
                eng.dma_start(t, src.ap())
                return t

            # dummy collective: absorbs the inter-core launch skew during the
            # startup DMA window so the first real AllToAll's barrier is cheap
            warm_in = dp.tile([1, 8], F32)
            warm_out = dp.tile([8, 8], F32, addr_space="Shared")
            nc.gpsimd.collective_compute(
                "AllGather", mybir.AluOpType.bypass,
                ins=[warm_in], outs=[warm_out], replica_groups=RG)
            # input x + embed weights first (they gate the first matmuls),
            # interleaved per feature-chunk and spread over both HWDGE
            # engines so dispatch (~0.5us per dma_start) isn't serialized
            xt_sb = mp.tile([128, 12, TPC], BF16, name="xt_sb")
            pw_sb = wp.tile([128, 12, 256], BF16, name="pw_sb")
            for i in range(12):
                eng = nc.sync if i % 2 == 0 else nc.scalar
                eng.dma_start(pw_sb[:, i:i + 1, :], pw_in.ap()[:, i:i + 1, :])
                eng.dma_start(xt_sb[:, i:i + 1, :], xt_in.ap()[:, i:i + 1, :])
            # phase A/B weights next; E/F weights are deferred (below) so they
            # don't compete for DMA bandwidth with the critical path
            tab_sb = wtile(tab_in, [128, 2, 128], BF16)
            iota_sb = wtile(iota_in, [128, 2])
            id_sb = wtile(id_in, [128, 128])
            clsv_sb = wtile(clsv_in, [1, 256], BF16)
            brow_sb = wtile(brow_in, [1, TPC], BF16)
            pb_sb = wtile(pb_in, [1, 256], BF16)
            wq_sb = wtile(wq_in, [128, 2, 256], BF16)
            bq_sb = wtile(bq_in, [128, 2])
            wk_sb = wtile(wk_in, [128, 2, 256], BF16)
            bk_sb = wtile(bk_in, [128, 2])
            wv_sb = wtile(wv_in, [128, 2, 256], BF16)
            bv_sb = wtile(bv_in, [128, 2])

            ones_row = wp.tile([1, 512], F32)
            nc.vector.memset(ones_row, 1.0)
            ones_row_bf = wp.tile([1, 512], BF16)
            nc.vector.memset(ones_row_bf, 1.0)
            e0_row = wp.tile([1, 512], BF16)
            nc.vector.memset(e0_row, 0.0)
            nc.vector.memset(e0_row[0:1, 0:1], 1.0)
            oinv = wp.tile([128, 1], F32)          # 1/256 for LN mean matmuls
            nc.vector.memset(oinv, 1.0 / 256.0)
            oinv_bf = wp.tile([128, 1], BF16)
            nc.vector.memset(oinv_bf, 1.0 / 256.0)
            eps1 = wp.tile([1, 1], F32)
            nc.vector.memset(eps1, 1e-5)
            id_bf = wp.tile([32, 32], BF16)
            nc.vector.tensor_copy(id_bf, id_sb[0:32, 0:32])
            id_bf128 = wp.tile([128, 128], BF16)
            nc.vector.tensor_copy(id_bf128, id_sb)

            # ---- big persistent activations ------------------------------
            h0T = mp.tile([128, 2, TPC], F32)

            # ============ phase A: embed + posemb =========================
            with tc.tile_pool(name="pA", bufs=1) as pa, \
                 tc.tile_pool(name="psA", bufs=1, space="PSUM") as psa:
                # coords -> one-hot grid-cell masks (bf16)
                oh = [[None, None], [None, None]]   # [coord][gchunk]
                for co in range(2):                 # 0: cg_row(=gh), 1: cg_col(=gw)
                    crd_sb = pa.tile([1, TPC], I32, name=f"crd{co}", tag="crd")
                    nc.gpsimd.dma_start(crd_sb, crd_in.ap()[co:co + 1, :])
                    cgi = pa.tile([1, TPC], I32, name=f"cgi{co}", tag="cgi")
                    nc.vector.tensor_scalar(cgi, crd_sb, 8, None,
                                            ALU.logical_shift_right)
                    cgf = pa.tile([1, TPC], BF16, name=f"cgf{co}", tag="cgf")
                    nc.vector.tensor_copy(cgf, cgi)
                    bc_ps = psa.tile([128, TPC], F32, name=f"bc{co}", tag="bc",
                                     bufs=2)
                    for t in range(2):
                        nc.tensor.matmul(bc_ps[:, 512 * t:512 * t + 512],
                                         ones_row_bf[0:1, 0:128],
                                         cgf[0:1, 512 * t:512 * t + 512],
                                         start=True, stop=True)
                    for gc in range(2):
                        o = pa.tile([128, TPC], BF16, name=f"oh{co}{gc}",
                                    tag=f"oh{co}{gc}")
                        nc.vector.tensor_scalar(o, bc_ps,
                                                iota_sb[:, gc:gc + 1], None,
                                                ALU.is_equal)
                        oh[co][gc] = o
                for oc in range(2):
                    for t in range(2):
                        hp = psa.tile([128, 512], F32, name=f"h0p{oc}{t}",
                                      tag="h0p", bufs=2)
                        for j in range(12):
                            nc.tensor.matmul(
                                hp, pw_sb[:, j, 128 * oc:128 * oc + 128],
                                xt_sb[:, j, 512 * t:512 * t + 512],
                                start=(j == 0), stop=False)
                        co = 1 - oc  # feats 0-127 use cg col (gw), 128-255 use cg row (gh)
                        for gc in range(2):
                            nc.tensor.matmul(
                                hp, tab_sb[:, gc, :],
                                oh[co][gc][:, 512 * t:512 * t + 512],
                                start=False, stop=False)
                        if t == 0:
                            nc.tensor.matmul(hp,
                                             clsv_sb[0:1, 128 * oc:128 * oc + 128],
                                             e0_row[0:1, 0:512],
                                             start=False, stop=False)
                        nc.tensor.matmul(hp, pb_sb[0:1, 128 * oc:128 * oc + 128],
                                         brow_sb[0:1, 512 * t:512 * t + 512],
                                         start=False, stop=True)
                        nc.vector.tensor_copy(h0T[:, oc, 512 * t:512 * t + 512],
                                              hp)
            if debug:
                nc.sync.dma_start(dbg["dbg_h0"].ap(),
                                  h0T.rearrange("p c t -> p (c t)"))

            # ============ LN helper (feature-major, full slice) ===========
            def layer_norm(src, dst, pool, psum, pfx, tA="lnA", tB="lnB"):
                sq = pool.tile([128, 2, TPC], BF16, name=pfx + "sq", tag="lnsq")
                for ch in range(2):
                    nc.vector.tensor_tensor(sq[:, ch], src[:, ch], src[:, ch],
                                            ALU.mult)
                sm_ps = psum.tile([1, TPC], F32, name=pfx + "sm", tag=tA)
                sq_ps = psum.tile([1, TPC], F32, name=pfx + "sqs", tag=tB)
                for t in range(2):
                    for ch in range(2):
                        nc.tensor.matmul(sm_ps[0:1, 512 * t:512 * t + 512], oinv,
                                         src[:, ch, 512 * t:512 * t + 512],
                                         start=(ch == 0), stop=(ch == 1))
                        nc.tensor.matmul(sq_ps[0:1, 512 * t:512 * t + 512],
                                         oinv_bf,
                                         sq[:, ch, 512 * t:512 * t + 512],
                                         start=(ch == 0), stop=(ch == 1))
                # scalar stats chain runs per 512-token half so the PE
                # broadcasts of half 0 overlap the chain of half 1
                mu = pool.tile([1, TPC], F32, name=pfx + "mu", tag="lnmu")
                msq = pool.tile([1, TPC], F32, name=pfx + "ms", tag="lnms")
                for t in range(2):
                    sl = slice(512 * t, 512 * t + 512)
                    nc.vector.tensor_copy(mu[:, sl], sm_ps[:, sl])
                    nc.vector.tensor_copy(msq[:, sl], sq_ps[:, sl])
                rb_ps = psum.tile([128, TPC], F32, name=pfx + "rb", tag=tA)
                db_ps = psum.tile([128, TPC], F32, name=pfx + "db", tag=tB)
                t1 = pool.tile([1, TPC], F32, name=pfx + "t1", tag="lnt1")
                var = pool.tile([1, TPC], F32, name=pfx + "var", tag="lnvar")
                sd = pool.tile([1, TPC], F32, name=pfx + "sd", tag="lnsd")
                rsig = pool.tile([1, TPC], F32, name=pfx + "rs", tag="lnrs")
                rsig_bf = pool.tile([1, TPC], BF16, name=pfx + "rsb", tag="lnrsb")
                dvec_bf = pool.tile([1, TPC], BF16, name=pfx + "dvb", tag="lndvb")
                for t in range(2):
                    # half 1's chain runs on the (idle) gpsimd engine so the
                    # two halves progress concurrently
                    eng = nc.vector if t == 0 else nc.gpsimd
                    sl = slice(512 * t, 512 * t + 512)
                    eng.tensor_tensor(t1[:, sl], mu[:, sl], mu[:, sl],
                                      ALU.mult)
                    eng.tensor_tensor(var[:, sl], msq[:, sl], t1[:, sl],
                                      ALU.subtract)
                    nc.scalar.activation(sd[:, sl], var[:, sl], AF.Sqrt,
                                         bias=eps1[0:1])
                    nc.vector.reciprocal_approx_fast(rsig[:, sl], sd[:, sl])
                    eng.tensor_copy(rsig_bf[:, sl], rsig[:, sl])
                    eng.tensor_tensor(dvec_bf[:, sl], mu[:, sl],
                                      rsig[:, sl], ALU.mult)
                    nc.tensor.matmul(rb_ps[:, sl], ones_row_bf[0:1, 0:128],
                                     rsig_bf[0:1, sl], start=True, stop=True)
                    nc.tensor.matmul(db_ps[:, sl], ones_row_bf[0:1, 0:128],
                                     dvec_bf[0:1, sl], start=True, stop=True)
                    for ch in range(2):
                        nc.vector.tensor_tensor(dst[:, ch, sl], src[:, ch, sl],
                                                rb_ps[:, sl], ALU.mult)
                        nc.vector.tensor_tensor(dst[:, ch, sl], dst[:, ch, sl],
                                                db_ps[:, sl], ALU.subtract)

            # ============ phase B: LN1 + local QKV + rotated AllToAll =====
            # Each core projects q/k/v for its own 1024 tokens (all heads),
            # then sends head j's 32-feature slice of q,k,v to core j with the
            # token axis cyclically rotated by j: on core c every received
            # block has column u holding global token 1024*j + (u+c)%1024, so
            # the dilation-r token class of head c sits at columns u = 0
            # (mod r) on EVERY core -- all attention APs become core-agnostic.
            ag_in = dp.tile([NCORES, 3, 32, TPC], BF16)
            ag_out = dp.tile([NCORES, 3, 32, TPC], BF16)
            with tc.tile_pool(name="pB", bufs=1) as pb_pool, \
                 tc.tile_pool(name="psB", bufs=1, space="PSUM") as psb:
                xh = pb_pool.tile([128, 2, TPC], BF16)
                layer_norm(h0T, xh, pb_pool, psb, "ln1")
                if debug:
                    nc.sync.dma_start(dbg["dbg_xh"].ap(),
                                      xh.rearrange("p c t -> p (c t)"))
                # local q/k/v projection: qkvL[:, 3*oc + ti, :]
                qkvL = pb_pool.tile([128, 6, TPC], BF16)
                for ti, (wt, bc) in [(2, (wv_sb, bv_sb)), (0, (wq_sb, bq_sb)),
                                     (1, (wk_sb, bk_sb))]:
                    for oc in range(2):
                        pp = psb.tile([128, TPC], F32, name=f"qkv{ti}{oc}",
                                      tag="qkvp", bufs=2)
                        for t in range(2):
                            sl = slice(512 * t, 512 * t + 512)
                            for ch in range(2):
                                nc.tensor.matmul(pp[:, sl],
                                                 wt[:, ch, 128 * oc:128 * oc + 128],
                                                 xh[:, ch, sl],
                                                 start=(ch == 0), stop=(ch == 1))
                        nc.vector.tensor_scalar(qkvL[:, 3 * oc + ti], pp,
                                                bc[:, oc:oc + 1], None, ALU.add)
                for j in range(NCORES):
                    # q, k and v ride in one DMA per destination to minimize
                    # dispatch cost (~0.5us per dma_start)
                    src = qkvL[32 * (j % 4):32 * (j % 4) + 32,
                               3 * (j // 4):3 * (j // 4) + 3, :]
                    dstv = ag_in[j].rearrange("t p c -> p t c")
                    eng = nc.sync if j % 2 == 0 else nc.scalar
                    eng.dma_start(dstv[:, :, 0:TPC - j], src[:, :, j:TPC])
                    if j:
                        eng.dma_start(dstv[:, :, TPC - j:TPC], src[:, :, 0:j])
                nc.gpsimd.collective_compute(
                    "AllToAll", mybir.AluOpType.bypass,
                    ins=[ag_in], outs=[ag_out], replica_groups=RG)

            # ============ phase C: assemble per-head q/k (4x) and v =======
            atp = tc.alloc_tile_pool(name="attp", bufs=1)
            qT4 = atp.tile([128, S], BF16)
            kT4 = atp.tile([128, S], BF16)
            with tc.tile_pool(name="pC1", bufs=1) as pc1, \
                 tc.tile_pool(name="psC", bufs=2, space="PSUM") as psc:
                vT = pc1.tile([32, S], BF16)
                for j in range(NCORES):
                    sl = slice(1024 * j, 1024 * j + 1024)
                    nc.gpsimd.dma_start(vT[:, sl], ag_out[j, 2, :, :])
                for j in range(NCORES):
                    sl = slice(1024 * j, 1024 * j + 1024)
                    nc.scalar.dma_start(qT4[0:32, sl], ag_out[j, 0, :, :])
                    nc.sync.dma_start(kT4[0:32, sl], ag_out[j, 1, :, :])
                for rep in (32, 64, 96):
                    for c2 in range(4):
                        cs = slice(2048 * c2, 2048 * c2 + 2048)
                        nc.scalar.dma_start(qT4[rep:rep + 32, cs],
                                            qT4[0:32, cs])
                        nc.sync.dma_start(kT4[rep:rep + 32, cs],
                                          kT4[0:32, cs])
                if debug:
                    nc.sync.dma_start(dbg["dbg_q"].ap(), qT4)
                    nc.sync.dma_start(dbg["dbg_k"].ap(), kT4)
                    nc.sync.dma_start(dbg["dbg_v"].ap(), vT)

                # ---- v33g: per-branch gathered token-major V + ones ------
                v33g = []
                for bi, r in enumerate(RATIOS):
                    n128 = (S // r if r < 16 else 512) // 128
                    vg = atp.tile([128, n128 * 33], BF16, name=f"v33g{bi}")
                    vg3 = vg.rearrange("p (j c) -> p j c", c=33)
                    nc.vector.memset(vg3[:, :, 32:33], 1.0)
                    v33g.append(vg)
                    for jg in range((n128 + 3) // 4):
                        nj = min(4, n128 - 4 * jg)
                        tp = psc.tile([128, 128], BF16, name=f"vg{bi}{jg}",
                                      tag="vgp", bufs=2)
                        for jj in range(nj):
                            j = 4 * jg + jj
                            if r == 1:
                                src = vT[:, 128 * j:128 * j + 128]
                            else:
                                src = vT.rearrange("p (t s) -> p t s", s=r)[
                                    :, 128 * j:128 * j + 128, 0]
                            nc.tensor.transpose(tp[:, 32 * jj:32 * jj + 32],
                                                src, id_bf)
                        nc.vector.tensor_copy(
                            vg3[:, 4 * jg:4 * jg + nj, 0:32],
                            tp.rearrange("p (j c) -> p j c", c=32)[:, 0:nj])

            # deferred weight loads (phase E/F) -- DMA bandwidth is free here
            wo_sb = wtile(wo_in, [128, 2, 256], BF16)
            bo_sb = wtile(bo_in, [128, 2])
            w1_sb = wtile(w1_in, [128, 2, FFN], BF16)
            b1_sb = wtile(b1_in, [128, 8])
            w2_sb = wtile(w2_in, [128, 8, 256], BF16)
            b2_sb = wtile(b2_in, [128, 2])
            sel_sb = wtile(sel_in, [8, 2, 128], BF16)
            hsel_sb = wtile(hsel_in, [128, 2, 8])
            sel64_sb = wtile(sel64_in, [64, 8])
            wk2_sb = wtile(wk2_in, [128, 2, 256], BF16)
            bk2_sb = wtile(bk2_in, [128, 2])
            wv2_sb = wtile(wv2_in, [128, 2, 256], BF16)
            bv2_sb = wtile(bv2_in, [128, 2])
            wq2_sb = wtile(wq2_in, [128, 2, 256])
            bq2_sb = wtile(bq2_in, [128, 2])
            wo2_sb = wtile(wo2_in, [128, 2, 256])
            bo2_sb = wtile(bo2_in, [128, 2])
            w12_sb = wtile(w12_in, [128, 2, FFN])
            b12_sb = wtile(b12_in, [128, 8])
            w22_sb = wtile(w22_in, [128, 8, 256])
            b22_sb = wtile(b22_in, [128, 2])
            m2_sb = wtile(m2_in, [8, TPC], BF16)
            encg_sb = wtile(encg_in, [128, 2])
            encb_sb = wtile(encb_in, [128, 2])
            nrmg_sb = wtile(nrmg_in, [128, 2])
            nrmb_sb = wtile(nrmb_in, [128, 2])

            # ============ phase D: dilated attention (row-tiled scores) ===
            # Columns of qT4/kT4/vT are in per-core rotated coordinates (see
            # phase B).  Scores contract over HD=32, so 4 key chunks run
            # concurrently in the PE array via tile_position row-tiling,
            # reading lhsT/rhs from the 4 q/k replicas at partitions 32i.
            # The exp (scalar engine) is the throughput limit; pv matmuls are
            # software-pipelined one iteration behind the scores.
            acc33 = atp.tile([128, 4096], F32, name="acc33")
            nc.vector.memset(acc33, 0.0)
            acc_bf = atp.tile([128, 4096], BF16, name="acc_bf")
            a2a_in = dp.tile([NCORES, 33, TPC], BF16)
            a2a_out = dp.tile([NCORES, 33, TPC], BF16)

            def stage_a2a_back(js):
                # the (num, den) accumulators travel in bf16 to halve wire time
                for j in js:
                    sl_p = slice(64 * (j // 4), 64 * (j // 4) + 33)
                    sl_t = slice(TPC * (j % 4), TPC * (j % 4) + TPC)
                    nc.vector.tensor_copy(acc_bf[sl_p, sl_t], acc33[sl_p, sl_t])
                    nc.sync.dma_start(a2a_in[j], acc_bf[sl_p, sl_t])

            # blocks ordered so all writers of acc33 window 0 (tokens 0..4095)
            # finish first: its a2a-back staging then overlaps window-1 compute
            def blk(bi, seg):
                return (bi, SEGMENTS[bi], RATIOS[bi], seg,
                        1024 if bi < 4 else 512)
            blocks_w0 = ([blk(0, s) for s in range(4)] +
                         [blk(1, 0), blk(1, 1), blk(2, 0), blk(3, 0),
                          blk(4, 0)])
            blocks_w1 = ([blk(0, s) for s in range(4, 8)] +
                         [blk(1, 2), blk(1, 3), blk(2, 1)])

            def drain_pv(pv_ps, r, seg, qh):
                q0 = 1024 * seg + 512 * qh
                CW = 4096 // r           # class tokens per acc33 group
                pos = 0
                while pos < 512:
                    i0 = q0 + pos
                    g = (r * i0) // 4096
                    li = i0 - g * CW
                    take = min(512 - pos, CW - (i0 % CW))
                    base = acc33[64 * g:64 * g + 33]
                    if r == 1:
                        aap = base[:, li:li + take]
                    else:
                        aap = base.rearrange("p (t s) -> p t s", s=r)[
                            :, li:li + take, 0]
                    nc.vector.tensor_tensor(
                        aap, pv_ps[0:33, pos:pos + take], aap, ALU.add)
                    pos += take

            with tc.tile_pool(name="ptp", bufs=2) as ptp, \
                 tc.tile_pool(name="psSC", bufs=3, space="PSUM") as pssc, \
                 tc.tile_pool(name="psPV", bufs=2, space="PSUM") as pspv:
                pending = [None]

                def flush_pv():
                    if pending[0] is None:
                        return
                    (pv_ps, bi_, Js, pt_, nkg_, kg_, r_, seg_, qh_) = pending[0]
                    for idx, J in enumerate(Js):
                        nc.tensor.matmul(
                            pv_ps[0:33, 0:512],
                            v33g[bi_][:, 33 * J:33 * J + 33],
                            pt_[:, 512 * idx:512 * idx + 512],
                            start=(kg_ == 0 and idx == 0),
                            stop=(kg_ == nkg_ - 1 and idx == 3))
                    if kg_ == nkg_ - 1:
                        drain_pv(pv_ps, r_, seg_, qh_)
                    pending[0] = None

                def run_blocks(blist):
                  for (bi, w, r, seg, cnt) in blist:
                    nq = cnt // 512
                    nkg = cnt // 512
                    for qh in range(nq):
                        pv_ps = pspv.tile([33, 512], F32, tag="pv")
                        for kg in range(nkg):
                            scA = pssc.tile([128, 1024], F32, tag="sc")
                            scB = pssc.tile([128, 1024], F32, tag="sc")
                            for i in range(4):
                                kc = 4 * kg + i
                                if r == 1:
                                    kap = kT4[32 * i:32 * i + 32,
                                              1024 * seg + 128 * kc:
                                              1024 * seg + 128 * kc + 128]
                                    qap = qT4[32 * i:32 * i + 32,
                                              1024 * seg + 512 * qh:
                                              1024 * seg + 512 * qh + 512]
                                else:
                                    kr = kT4[32 * i:32 * i + 32].rearrange(
                                        "p (t s) -> p t s", s=r)
                                    qr = qT4[32 * i:32 * i + 32].rearrange(
                                        "p (t s) -> p t s", s=r)
                                    kap = kr[:, 1024 * seg + 128 * kc:
                                             1024 * seg + 128 * kc + 128, 0]
                                    qap = qr[:, 1024 * seg + 512 * qh:
                                             1024 * seg + 512 * qh + 512, 0]
                                dst = (scA if i < 2 else scB)[
                                    :, 512 * (i % 2):512 * (i % 2) + 512]
                                nc.tensor.matmul(dst, kap, qap,
                                                 start=True, stop=True,
                                                 tile_position=(32 * i, 0))
                            pt = ptp.tile([128, 2048], BF16, tag="pt")
                            nc.scalar.activation(pt[:, 0:1024], scA, AF.Exp,
                                                 scale=SCALE)
                            nc.scalar.activation(pt[:, 1024:2048], scB, AF.Exp,
                                                 scale=SCALE)
                            flush_pv()
                            Js = [8 * seg + 4 * kg + i for i in range(4)]
                            pending[0] = (pv_ps, bi, Js, pt, nkg, kg, r, seg, qh)

                run_blocks(blocks_w0)
                flush_pv()
                stage_a2a_back(range(4))
                run_blocks(blocks_w1)
                flush_pv()
                stage_a2a_back(range(4, 8))
            if debug:
                nc.sync.dma_start(dbg["dbg_acc"].ap(), acc33)

            # ============ phase E: AllToAll + normalize + Wo + FFN ========
            nc.gpsimd.collective_compute(
                "AllToAll", mybir.AluOpType.bypass,
                ins=[a2a_in], outs=[a2a_out], replica_groups=RG)
            atp.release()

            h1T = mp.tile([128, 2, TPC], F32)
            h2T = mp.tile([128, 2, TPC], F32)
            agc_in = dp.tile([128, 2], F32)
            agc_out = dp.tile([128 * NCORES, 2], F32, addr_space="Shared")
            with tc.tile_pool(name="pE", bufs=1) as pe, \
                 tc.tile_pool(name="pEg", bufs=2) as peg, \
                 tc.tile_pool(name="psE", bufs=1, space="PSUM") as pse, \
                 tc.tile_pool(name="psEf", bufs=2, space="PSUM") as psef:
                attnT = pe.tile([128, 2, TPC], BF16)
                den_bf = pe.tile([8, TPC], BF16)
                # den rows first so the reciprocal + broadcast overlap the
                # (larger) numerator un-rotate DMAs
                for h in range(NCORES):
                    eng = nc.scalar if h % 2 == 0 else nc.sync
                    eng.dma_start(den_bf[h:h + 1, h:TPC],
                                  a2a_out[h, 32:33, 0:TPC - h])
                    if h:
                        eng.dma_start(den_bf[h:h + 1, 0:h],
                                      a2a_out[h, 32:33, TPC - h:TPC])
                for h in range(NCORES):
                    # un-rotate by the sending head h: local token v came from
                    # column (v - h) % 1024 of head-core h's accumulator
                    psl = slice(32 * (h % 4), 32 * (h % 4) + 32)
                    eng = nc.sync if h % 2 == 0 else nc.scalar
                    eng.dma_start(attnT[psl, h // 4, h:TPC],
                                  a2a_out[h, 0:32, 0:TPC - h])
                    if h:
                        eng.dma_start(attnT[psl, h // 4, 0:h],
                                      a2a_out[h, 0:32, TPC - h:TPC])
                den = pe.tile([8, TPC], F32)
                nc.vector.tensor_copy(den, den_bf)
                rec = pe.tile([8, TPC], F32)
                nc.vector.reciprocal_approx_fast(rec, den)
                rec_bf = pe.tile([8, TPC], BF16)
                nc.vector.tensor_copy(rec_bf, rec)
                attnO = pe.tile([128, 2, TPC], BF16)
                for ch in range(2):
                    rb_ps = pse.tile([128, TPC], F32, name=f"rb{ch}", tag="eA")
                    for t in range(2):
                        nc.tensor.matmul(rb_ps[:, 512 * t:512 * t + 512],
                                         sel_sb[:, ch, :],
                                         rec_bf[:, 512 * t:512 * t + 512],
                                         start=True, stop=True)
                    nc.vector.tensor_tensor(attnO[:, ch], attnT[:, ch], rb_ps,
                                            ALU.mult)
                if debug:
                    nc.sync.dma_start(dbg["dbg_att"].ap(),
                                      attnO.rearrange("p c t -> p (c t)"))
                # Wo + residual (bias fused into the drain)
                for oc in range(2):
                    hp = pse.tile([128, TPC], F32, name=f"h1p{oc}", tag="eA")
                    for t in range(2):
                        sl = slice(512 * t, 512 * t + 512)
                        for ch in range(2):
                            nc.tensor.matmul(hp[:, sl],
                                             wo_sb[:, ch, 128 * oc:128 * oc + 128],
                                             attnO[:, ch, sl],
                                             start=(ch == 0), stop=(ch == 1))
                    nc.vector.scalar_tensor_tensor(
                        h1T[:, oc], hp, bo_sb[:, oc:oc + 1], h0T[:, oc],
                        ALU.add, ALU.add)
                if debug:
                    nc.sync.dma_start(dbg["dbg_h1"].ap(),
                                      h1T.rearrange("p c t -> p (c t)"))
                # LN2 + FFN
                xh2 = pe.tile([128, 2, TPC], BF16)
                layer_norm(h1T, xh2, pe, pse, "ln2", tA="eA", tB="eB")
                h2ps = []
                for oc in range(2):
                    h2ps.append(pse.tile([128, TPC], F32, name=f"h2p{oc}",
                                         tag=("eA" if oc == 0 else "eB")))
                for fc in range(8):
                    fp = psef.tile([128, TPC], F32, tag="f1")
                    for t in range(2):
                        sl = slice(512 * t, 512 * t + 512)
                        for ch in range(2):
                            nc.tensor.matmul(fp[:, sl],
                                             w1_sb[:, ch, 128 * fc:128 * fc + 128],
                                             xh2[:, ch, sl],
                                             start=(ch == 0), stop=(ch == 1))
                    g = peg.tile([128, TPC], BF16, tag="gel")
                    nc.scalar.activation(g, fp, AF.Gelu,
                                         bias=b1_sb[:, fc:fc + 1])
                    for oc in range(2):
                        for t in range(2):
                            sl = slice(512 * t, 512 * t + 512)
                            nc.tensor.matmul(h2ps[oc][:, sl],
                                             w2_sb[:, fc, 128 * oc:128 * oc + 128],
                                             g[:, sl],
                                             start=(fc == 0),
                                             stop=(fc == 7))
                # drain t=0 halves first so the cls AllGather launches early
                for t in range(2):
                    sl = slice(512 * t, 512 * t + 512)
                    for oc in range(2):
                        nc.vector.scalar_tensor_tensor(
                            h2T[:, oc, sl], h2ps[oc][:, sl],
                            b2_sb[:, oc:oc + 1], h1T[:, oc, sl],
                            ALU.add, ALU.add)
                    if t == 0:
                        nc.sync.dma_start(
                            agc_in,
                            h2T[:, :, 0:1].rearrange("p c o -> p (c o)"))
                        nc.gpsimd.collective_compute(
                            "AllGather", mybir.AluOpType.bypass,
                            ins=[agc_in], outs=[agc_out], replica_groups=RG)
            if debug:
                nc.sync.dma_start(dbg["dbg_h2"].ap(),
                                  h2T.rearrange("p c t -> p (c t)"))

            # ============ phase F: layer 2 (cls query only) ===============
            agp_in = dp.tile([8, 257], F32)
            agp_out = dp.tile([8 * NCORES, 257], F32, addr_space="Shared")

            with tc.tile_pool(name="pF", bufs=1) as pf, \
                 tc.tile_pool(name="pFs", bufs=2) as pfs, \
                 tc.tile_pool(name="psF", bufs=1, space="PSUM") as psf:
                xh3 = pf.tile([128, 2, TPC], BF16)
                layer_norm(h2T, xh3, pf, psf, "ln12", tA="fA", tB="fB")
                k2T = pf.tile([128, 2, TPC], BF16)
                v2T = pf.tile([128, 2, TPC], BF16)
                for (wt, bt, dst) in [(wk2_sb, bk2_sb, k2T),
                                      (wv2_sb, bv2_sb, v2T)]:
                    for oc in range(2):
                        pp = psf.tile([128, TPC], F32, name=f"kv2_{oc}",
                                      tag=("fA" if oc == 0 else "fB"))
                        for t in range(2):
                            sl = slice(512 * t, 512 * t + 512)
                            for ch in range(2):
                                nc.tensor.matmul(pp[:, sl],
                                                 wt[:, ch, 128 * oc:128 * oc + 128],
                                                 xh3[:, ch, sl],
                                                 start=(ch == 0), stop=(ch == 1))
                        nc.vector.tensor_scalar(dst[:, oc], pp,
                                                bt[:, oc:oc + 1], None, ALU.add)
                # token-major v2 with a ones column for the denominator
                v2t257 = pf.tile([128, 8, 257], BF16)
                nc.vector.memset(v2t257[:, :, 256:257], 1.0)
                for tc8 in range(8):
                    tp = psf.tile([128, 256], BF16, name=f"v2t{tc8}",
                                  tag=("fC" if tc8 % 2 == 0 else "fD"))
                    for ch in range(2):
                        nc.tensor.transpose(tp[:, 128 * ch:128 * ch + 128],
                                            v2T[:, ch, 128 * tc8:128 * tc8 + 128],
                                            id_bf128)
                    nc.vector.tensor_copy(v2t257[:, tc8, 0:256], tp)

                # cls row: LN + q2
                h2c = pf.tile([128, 2], F32)
                nc.sync.dma_start(h2c, agc_out[0:128, :])
                x3c = pf.tile([128, 2], F32)
                nc.gpsimd.layernorm(x3c, h2c, subtract_mean=True)
                q2_ps = psf.tile([128, 2], F32, name="q2ps", tag="fC")
                for oc in range(2):
                    for ch in range(2):
                        nc.tensor.matmul(q2_ps[:, oc:oc + 1],
                                         wq2_sb[:, ch, 128 * oc:128 * oc + 128],
                                         x3c[:, ch:ch + 1],
                                         start=(ch == 0), stop=(ch == 1))
                q2 = pf.tile([128, 2], F32)
                nc.vector.tensor_tensor(q2, q2_ps, bq2_sb, ALU.add)
                q2b = pf.tile([128, 2, 8], BF16)
                nc.vector.memset(q2b, 0.0)
                for h in range(8):
                    nc.vector.tensor_copy(
                        q2b[32 * (h % 4):32 * (h % 4) + 32, h // 4, h:h + 1],
                        q2[32 * (h % 4):32 * (h % 4) + 32, h // 4:h // 4 + 1])

                # scores in head-major layout: s2T[h, key]
                s2_ps = psf.tile([8, TPC], F32, name="s2ps", tag="fD")
                for t in range(2):
                    sl = slice(512 * t, 512 * t + 512)
                    for ch in range(2):
                        nc.tensor.matmul(s2_ps[0:8, sl], q2b[:, ch, :],
                                         k2T[:, ch, sl],
                                         start=(ch == 0), stop=(ch == 1))
                p2 = pf.tile([8, TPC], BF16)
                nc.scalar.activation(p2, s2_ps, AF.Exp, scale=SCALE)
                p2m = pf.tile([8, TPC], BF16)
                nc.vector.tensor_tensor(p2m, p2, m2_sb, ALU.mult)
                # transpose p2m to token-major [128, 8, 8]
                pT_ps = psf.tile([128, 64], BF16, name="pTps", tag="fC")
                for c8 in range(8):
                    nc.tensor.transpose(pT_ps[:, 8 * c8:8 * c8 + 8],
                                        p2m[:, 128 * c8:128 * c8 + 128],
                                        id_bf[0:8, 0:8])
                p2mT = pf.tile([128, 8, 8], BF16)
                nc.vector.tensor_copy(
                    p2mT.rearrange("p a b -> p (a b)"), pT_ps)
                # num | den in one accumulation: out[h, 0:256]=num, [h,256]=den
                oad_ps = psf.tile([8, 257], F32, name="oadps", tag="fD")
                for c8 in range(8):
                    nc.tensor.matmul(oad_ps, p2mT[:, c8, :],
                                     v2t257[:, c8, :],
                                     start=(c8 == 0), stop=(c8 == 7))
                part = pf.tile([8, 257], F32)
                nc.vector.tensor_copy(part, oad_ps)
                nc.sync.dma_start(agp_in, part)
                nc.gpsimd.collective_compute(
                    "AllGather", mybir.AluOpType.bypass,
                    ins=[agp_in], outs=[agp_out], replica_groups=RG)

                # ---- final combine (every core computes the full answer) --
                agp_sb = pf.tile([64, 257], F32)
                nc.sync.dma_start(agp_sb, agp_out[0:64, :])
                na_ps = psf.tile([8, 257], F32, name="naps", tag="fE")
                nc.tensor.matmul(na_ps, sel64_sb, agp_sb,
                                 start=True, stop=True)
                na = pf.tile([8, 257], F32)
                nc.vector.tensor_copy(na, na_ps)
                rec2 = pf.tile([8, 1], F32)
                nc.vector.reciprocal_approx_fast(rec2, na[:, 256:257])
                o2v = pf.tile([8, 256], F32)
                nc.vector.tensor_scalar(o2v, na[:, 0:256], rec2, None, ALU.mult)
                # head-major -> feature-column, pick head h = feat // 32
                o2T_ps = psf.tile([128, 16], F32, name="o2Tps", tag="fC")
                for ch in range(2):
                    nc.tensor.transpose(o2T_ps[:, 8 * ch:8 * ch + 8],
                                        o2v[0:8, 128 * ch:128 * ch + 128],
                                        id_sb[0:8, 0:8])
                o2m = pf.tile([128, 2, 8], F32)
                nc.vector.tensor_tensor(
                    o2m.rearrange("p a b -> p (a b)"), o2T_ps,
                    hsel_sb.rearrange("p a b -> p (a b)"), ALU.mult)
                o2c = pf.tile([128, 2], F32)
                nc.vector.tensor_reduce(o2c, o2m, mybir.AxisListType.X,
                                        ALU.add)
                # Wo2 + residual
                o3_ps = psf.tile([128, 2], F32, name="o3ps", tag="fD")
                for oc in range(2):
                    for ch in range(2):
                        nc.tensor.matmul(o3_ps[:, oc:oc + 1],
                                         wo2_sb[:, ch, 128 * oc:128 * oc + 128],
                                         o2c[:, ch:ch + 1],
                                         start=(ch == 0), stop=(ch == 1))
                h3 = pf.tile([128, 2], F32)
                nc.vector.tensor_tensor(h3, o3_ps, bo2_sb, ALU.add)
                nc.vector.tensor_tensor(h3, h3, h2c, ALU.add)
                # LN2_2 + FFN row
                x4 = pf.tile([128, 2], F32)
                nc.gpsimd.layernorm(x4, h3, subtract_mean=True)
                f1_ps = psf.tile([128, 8], F32, name="f1ps", tag="fC")
                for fc in range(8):
                    for ch in range(2):
                        nc.tensor.matmul(f1_ps[:, fc:fc + 1],
                                         w12_sb[:, ch, 128 * fc:128 * fc + 128],
                                         x4[:, ch:ch + 1],
                                         start=(ch == 0), stop=(ch == 1))
                f1b = pf.tile([128, 8], F32)
                nc.vector.tensor_tensor(f1b, f1_ps, b12_sb, ALU.add)
                g2 = pf.tile([128, 8], F32)
                nc.scalar.activation(g2, f1b, AF.Gelu)
                h4_ps = psf.tile([128, 2], F32, name="h4ps", tag="fD")
                for oc in range(2):
                    for fc in range(8):
                        nc.tensor.matmul(h4_ps[:, oc:oc + 1],
                                         w22_sb[:, fc, 128 * oc:128 * oc + 128],
                                         g2[:, fc:fc + 1],
                                         start=(fc == 0), stop=(fc == 7))
                h4 = pf.tile([128, 2], F32)
                nc.vector.tensor_tensor(h4, h4_ps, b22_sb, ALU.add)
                nc.vector.tensor_tensor(h4, h4, h3, ALU.add)
                h5 = pf.tile([128, 2], F32)
                nc.gpsimd.layernorm(h5, h4, gamma_ap=encg_sb, beta_ap=encb_sb,
                                    subtract_mean=True)
                h6 = pf.tile([128, 2], F32)
                nc.gpsimd.layernorm(h6, h5, gamma_ap=nrmg_sb, beta_ap=nrmb_sb,
                                    subtract_mean=True)
                ot_ps = psf.tile([2, 128], F32, name="otps", tag="fC")
                nc.tensor.transpose(ot_ps, h6, id_sb)
                ot = pf.tile([2, 128], F32)
                nc.vector.tensor_copy(ot, ot_ps)
                nc.sync.dma_start(out_d.ap(), ot)

    nc.compile()
    return nc


# ----------------------------------------------------------------------------
# host-side input packing
# ----------------------------------------------------------------------------
def _f(a):
    return np.ascontiguousarray(np.asarray(a, dtype=np.float32))


def prep_in_maps(inputs):
    x = _f(inputs["x"]).reshape(L, D_IN)
    coords = np.asarray(inputs["coords"]).reshape(L, 2).astype(np.int32)
    proj_w = _f(inputs["proj_w"]); proj_b = _f(inputs["proj_b"])
    cls_tok = _f(inputs["cls_tok"]).reshape(256)
    Wq = _f(inputs["Wq"]); Wk = _f(inputs["Wk"]); Wv = _f(inputs["Wv"])
    Wo = _f(inputs["Wo"])
    bq = _f(inputs["bq"]); bk = _f(inputs["bk"]); bv = _f(inputs["bv"])
    bo = _f(inputs["bo"])
    ln1_g = _f(inputs["ln1_g"]); ln1_b = _f(inputs["ln1_b"])
    ln2_g = _f(inputs["ln2_g"]); ln2_b = _f(inputs["ln2_b"])
    W1 = _f(inputs["W1"]); b1 = _f(inputs["b1"])
    W2 = _f(inputs["W2"]); b2 = _f(inputs["b2"])
    enc_g = _f(inputs["enc_g"]); enc_b = _f(inputs["enc_b"])
    norm_g = _f(inputs["norm_g"]); norm_b = _f(inputs["norm_b"])

    # constants
    om = 1.0 / (10000.0 ** (np.arange(64, dtype=np.float64) / 64.0))
    g = np.arange(256, dtype=np.float64)[:, None] * om[None, :]
    tab = np.concatenate([np.sin(g), np.cos(g)], 1).astype(np.float32)  # [256,128]
    tab_l = np.ascontiguousarray(tab.reshape(2, 128, 128).transpose(1, 0, 2))
    iota2 = np.arange(256, dtype=np.float32).reshape(2, 128).T.copy()
    id128 = np.eye(128, dtype=np.float32)
    selm = np.zeros((8, 2, 128), np.float32)
    for j in range(8):
        for f in range(256):
            if f // 32 == j:
                selm[j, f // 128, f % 128] = 1.0
    hsel = np.zeros((128, 2, 8), np.float32)
    for ch in range(2):
        for p in range(128):
            hsel[p, ch, (128 * ch + p) // 32] = 1.0
    sel64 = np.zeros((64, 8), np.float32)
    for j in range(8):
        for h in range(8):
            sel64[8 * j + h, h] = 1.0

    def lhsT_chunks(w, nch):   # [Din, Dout] -> [128, nch, Dout]
        return np.ascontiguousarray(
            w.reshape(nch, 128, w.shape[1]).transpose(1, 0, 2))

    def col2(b):               # [256] -> [128, 2] feature-column layout
        return np.ascontiguousarray(b.reshape(2, 128).T)

    # layer-1 folds
    wq_e = ln1_g[0][:, None] * Wq[0]
    wk_e = ln1_g[0][:, None] * Wk[0]
    wv_e = ln1_g[0][:, None] * Wv[0]
    bq_e = bq[0] + ln1_b[0] @ Wq[0]
    bk_e = bk[0] + ln1_b[0] @ Wk[0]
    bv_e = bv[0] + ln1_b[0] @ Wv[0]
    w1_e = ln2_g[0][:, None] * W1[0]
    b1_e = b1[0] + ln2_b[0] @ W1[0]
    # layer-2 folds
    wq2_e = ln1_g[1][:, None] * Wq[1]
    wk2_e = ln1_g[1][:, None] * Wk[1]
    wv2_e = ln1_g[1][:, None] * Wv[1]
    bq2_e = bq[1] + ln1_b[1] @ Wq[1]
    bk2_e = bk[1] + ln1_b[1] @ Wk[1]
    bv2_e = bv[1] + ln1_b[1] @ Wv[1]
    w12_e = ln2_g[1][:, None] * W1[1]
    b12_e = b1[1] + ln2_b[1] @ W1[1]

    shared = {
        "tab": tab_l.astype(BF), "iota2": iota2, "id128": id128,
        "selm": selm.astype(BF), "hsel": hsel, "sel64": sel64,
        "pw": lhsT_chunks(proj_w, 12).astype(BF),
        "pb": proj_b.reshape(1, 256).astype(BF),
        "wqf": lhsT_chunks(wq_e, 2).astype(BF), "bqc": col2(bq_e),
        "wkf": lhsT_chunks(wk_e, 2).astype(BF), "bkc": col2(bk_e),
        "wvf": lhsT_chunks(wv_e, 2).astype(BF), "bvc": col2(bv_e),
        "wo": lhsT_chunks(Wo[0], 2).astype(BF), "boc": col2(bo[0]),
        "w1": lhsT_chunks(w1_e, 2).astype(BF),
        "b1c": np.ascontiguousarray(b1_e.reshape(8, 128).T),
        "w2": lhsT_chunks(W2[0], 8).astype(BF), "b2c": col2(b2[0]),
        "wk2": lhsT_chunks(wk2_e, 2).astype(BF), "bk2c": col2(bk2_e),
        "wv2": lhsT_chunks(wv2_e, 2).astype(BF), "bv2c": col2(bv2_e),
        "wq2": lhsT_chunks(wq2_e, 2), "bq2c": col2(bq2_e),
        "wo2": lhsT_chunks(Wo[1], 2), "bo2c": col2(bo[1]),
        "w12": lhsT_chunks(w12_e, 2),
        "b12c": np.ascontiguousarray(b12_e.reshape(8, 128).T),
        "w22": lhsT_chunks(W2[1], 8), "b22c": col2(b2[1]),
        "encg": col2(enc_g), "encb": col2(enc_b),
        "nrmg": col2(norm_g), "nrmb": col2(norm_b),
    }
    shared = {k: np.ascontiguousarray(v) for k, v in shared.items()}

    in_maps = []
    for c in range(NCORES):
        x_sl = np.zeros((TPC, D_IN), np.float32)
        crd = np.full((TPC, 2), -256, np.int32)
        if c == 0:
            x_sl[1:] = x[0:TPC - 1]
            crd[1:] = coords[0:TPC - 1]
        else:
            x_sl[:] = x[TPC * c - 1:TPC * (c + 1) - 1]
            crd[:] = coords[TPC * c - 1:TPC * (c + 1) - 1]
        # feature-major x: [128, 12, TPC]
        xt = np.ascontiguousarray(
            x_sl.T.reshape(12, 128, TPC).transpose(1, 0, 2)).astype(BF)
        brow = np.ones((1, TPC), np.float32)
        clsv = np.zeros((1, 256), np.float32)
        if c == 0:
            brow[0, 0] = 0.0
            clsv[0] = cls_tok
        # layer-2 multiplicity mask  m[h, j_local]
        jj = TPC * c + np.arange(TPC)
        m2 = np.zeros((8, TPC), np.float32)
        for h in range(8):
            for w, r in zip(SEGMENTS, RATIOS):
                if h % r == 0:
                    m2[h] += ((jj % r == 0) & (jj < w)).astype(np.float32)
        d = dict(shared)
        d.update({
            "xt": xt, "crd": np.ascontiguousarray(crd.T),
            "brow": brow.astype(BF), "clsv": clsv.astype(BF),
            "m2r": m2.astype(BF),
        })
        in_maps.append(d)
    return in_maps


def get_program(debug=False):
    key = ("dbg" if debug else "std")
    if key not in _CACHE:
        _CACHE[key] = build_program(debug=debug)
    return _CACHE[key]


def run(inputs, debug=False, trace=False, **kw):
    from concourse import bass_utils
    nc = get_program(debug=debug)
    in_maps = prep_in_maps(inputs)
    res = bass_utils.run_bass_kernel_spmd(
        nc, in_maps, core_ids=list(range(NCORES)), trace=trace, **kw)
    return res


def kernel(**inputs):
    res = run(inputs)
    out = res.results[0]["out"]          # [2, 128] feature-chunk layout
    return np.ascontiguousarray(out.reshape(1, 256))
